# revision 28
# baseline (speedup 1.0000x reference)
"""Differential attention (nn_DifferentialAttention_84679575208071) on 8 TRN2
NeuronCores via Bass/Tile.

Sharding: hybrid data-parallel x tensor-parallel. Core c handles batch c//4 and
heads 4*(c%4) .. 4*(c%4)+4 (B=2, H=16 -> 2 batch groups x 4-way head split).
Each core computes its 4 heads' attention + group-norm + partial o_proj; the
host sums the 4 partial (S, D) outputs per batch and stacks the 2 batches.

v5 design (cost-model driven):
  - The host pre-transposes x and packs ALL inputs (x^T, weights, rope tables,
    identity/rotation matrices) into 8 need-ordered bf16 column bundles, one
    DMA each.  Every DMA costs ~3us of queue-head ring-semaphore blocking, so
    transfer count -- not bytes -- dominates the prologue; with 8 bundles the
    first exp fires at ~11us instead of ~59us.
  - Attention PV matmuls reoriented: E (bf16, SBUF) is the stationary operand,
    aug-V [64 cols + ones] the moving one -> 65-column matmuls, and the output
    lands token-major [q, ch], which is what group-norm wants.
  - Per-(sk, branch) pipeline: QK [128,512] -> exp -> 4 PV through a PSUM
    ring; the next block's first QK/exp pair is emitted ahead of the current
    block's last PV so the ACT engine (the ~220us exp floor) never waits
    across block boundaries.
  - PV accumulators double-buffered by block parity so a new block's PV
    matmuls never wait on the previous block's group-norm drain.
  - Projections and o_proj split into half-units and spread over 14 per-block
    feed slots so PE bursts never outrun the exp buffer.
"""
import json
import time

import numpy as np

import bass_rust
import concourse.bass as bass
import concourse.mybir as mybir
import concourse.tile as tile

F32 = mybir.dt.float32
BF16 = mybir.dt.bfloat16
MULT = mybir.AluOpType.mult
ADD = mybir.AluOpType.add
SUB = mybir.AluOpType.subtract
AX = mybir.AxisListType.X
AF = mybir.ActivationFunctionType

B, S, D = 2, 2048, 1024
H, Hd = 16, 64
HPC = 4               # heads per core
THDC = HPC * Hd       # 256 channels per core
NT = S // 128         # 16 s-tiles
NCH = S // 512        # 4 chunks
DK = D // 128         # 8 contraction tiles
LAMBDA_INIT = 0.8
GN_EPS = 1e-5
ROPE_BASE = 10000.0
QK_SCALE = float(Hd) ** -0.5  # 0.125

# bund1 column offsets (bf16): xT-c0 | wk02 | wq02 | cos0 | sin0 | rotm | ident
B1_XT = 0             # 8 dk x 512
B1_WK = 4096          # 8 dk x (tt0 128 | tt2 128)
B1_WQ = 6144
B1_COS = 8192
B1_SIN = 8704
B1_ROTM = 9216
B1_ID = 9344
B1_W = 9472
# bundc (c=1,2,3): xT-c | cos-c | sin-c
BC_COS = 4096
BC_SIN = 4608
BC_W = 5120
# bund5: wk13 | wq13 | wo | gnw_eff | gnb_eff | lam_bh
B5_WK = 0
B5_WQ = 2048
B5_WO = 4096
B5_GNW = 6144
B5_GNB = 6400
B5_LAM = 6656
B5_W = 6672


def _split_multi_waits(nc):
    """This container's walrus rejects >1 sync wait per instruction. Hoist
    extra waits onto same-engine NoOps inserted right before the instruction
    (engine queues are in-order, so semantics are unchanged)."""
    d = json.loads(bass_rust.module_to_json_string(nc.m))
    ctr = 0
    for f in d["functions"]:
        for bb in f["blocks"]:
            out = []
            for inst in bb.get("instructions", []):
                si = inst.get("sync_info")
                waits = si.get("on_wait", []) if si else []
                if len(waits) > 1:
                    for w in waits[:-1]:
                        ctr += 1
                        out.append({
                            "debug": inst.get("debug", 0),
                            "engine": inst["engine"],
                            "ins": [],
                            "name": f"WSPLIT-{ctr}",
                            "opcode": "NoOp",
                            "outs": [],
                            "sync_info": {"on_update": [], "on_wait": [w]},
                        })
                    si["on_wait"] = [waits[-1]]
                out.append(inst)
            bb["instructions"] = out
    nc.m = bass_rust.module_from_json_string(json.dumps(d))


def _rope_tables():
    inv_freq = 1.0 / (ROPE_BASE ** (np.arange(0, Hd, 2, dtype=np.float32) / Hd))
    t = np.arange(S, dtype=np.float32)
    freqs = np.outer(t, inv_freq).astype(np.float32)
    emb = np.concatenate([freqs, freqs], axis=-1)       # (S, Hd)
    cos = np.cos(emb).astype(np.float32)
    sin = np.sin(emb).astype(np.float32)
    return cos.T.copy(), sin.T.copy()                   # (Hd, S) each


def _rotm():
    m64 = np.zeros((64, 64), dtype=np.float32)
    for d_ in range(32):
        m64[d_, d_ + 32] = -1.0
        m64[d_ + 32, d_] = 1.0
    m128 = np.zeros((128, 128), dtype=np.float32)
    m128[:64, :64] = m64
    m128[64:, 64:] = m64
    return np.ascontiguousarray(m128.T)


def _hview(t):
    return t[:, :].rearrange("p (g c) -> p g c", g=HPC)[:, :, 0:Hd]


def _rview(t):
    v = t[:, :].rearrange("p (g c) -> p g c", g=HPC)[:, :, Hd:Hd + 1]
    return v.rearrange("p g c -> p (g c)")


def _b64(ap):
    return ap.unsqueeze(2).broadcast_to((128, HPC, Hd))


def build_module():
    nc = bass.Bass(trn_type="TRN2")

    b1_d = nc.dram_tensor("b1", [128, B1_W], BF16, kind="ExternalInput")
    wv_d = nc.dram_tensor("wvb", [128, 2048], BF16, kind="ExternalInput")
    bc_d = [nc.dram_tensor(f"bc{c}", [128, BC_W], BF16, kind="ExternalInput")
            for c in (1, 2, 3)]
    b5_d = nc.dram_tensor("b5", [128, B5_W], BF16, kind="ExternalInput")
    y_d = nc.dram_tensor("y", [S, D], F32, kind="ExternalOutput")

    dma_sp = nc.sync.dma_start

    with tile.TileContext(nc) as tc:
        with tc.tile_pool(name="persist", bufs=1) as pA:
            # ---- 8 input DMAs on the SP queue, in need order ----
            b1 = pA.tile([128, B1_W], BF16, tag="b1", name="b1")
            dma_sp(out=b1, in_=b1_d.ap())
            wvb = pA.tile([128, 2048], BF16, tag="wvb", name="wvb")
            dma_sp(out=wvb, in_=wv_d.ap())
            bc = [None]
            for i in range(3):
                t = pA.tile([128, BC_W], BF16, tag=f"bc{i}", name=f"bc{i}")
                dma_sp(out=t, in_=bc_d[i].ap())
                bc.append(t)
            b5 = pA.tile([128, B5_W], BF16, tag="b5", name="b5")
            dma_sp(out=b5, in_=b5_d.ap())

            rotm = b1[:, B1_ROTM:B1_ROTM + 128]
            ident = b1[:, B1_ID:B1_ID + 128]

            def xt(dk, c):
                t = b1 if c == 0 else bc[c]
                return t[:, dk * 512:(dk + 1) * 512]

            def wkq_sl(which, tt, dk):
                # which 0=K 1=Q
                if tt in (0, 2):
                    base = (B1_WK if which == 0 else B1_WQ)
                    return b1[:, base + dk * 256 + (tt // 2) * 128:
                              base + dk * 256 + (tt // 2) * 128 + 128]
                base = (B5_WK if which == 0 else B5_WQ)
                return b5[:, base + dk * 256 + (tt // 2) * 128:
                          base + dk * 256 + (tt // 2) * 128 + 128]

            def wv_sl(dk):
                return wvb[:, dk * THDC:(dk + 1) * THDC]

            def wo_sl(ci, oc):
                return b5[:, B5_WO + ci * 1024 + oc * 512:B5_WO + ci * 1024 + (oc + 1) * 512]

            def cos_sl(c):
                t = b1 if c == 0 else bc[c]
                off = B1_COS if c == 0 else BC_COS
                return t[:, off:off + 512]

            def sin_sl(c):
                t = b1 if c == 0 else bc[c]
                off = B1_SIN if c == 0 else BC_SIN
                return t[:, off:off + 512]

            # lambda / group-norm affine constants are host-precomputed
            # bf16 broadcast rows packed into b5.
            gnw_eff = b5[:, B5_GNW:B5_GNW + THDC]
            gnb_eff = b5[:, B5_GNB:B5_GNB + THDC]
            lam_bh = [b5[:, B5_LAM + 4 * h:B5_LAM + 4 * (h + 1)] for h in range(HPC)]

            # persistent compute tensors
            dummy = pA.tile([128, 512], BF16, tag="dummy", name="dummy")
            nc.vector.memset(dummy, 0.5)
            qt = [pA.tile([128, S], BF16, tag=f"qt{i}", name=f"qt{i}") for i in range(4)]
            kt = [pA.tile([128, S], BF16, tag=f"kt{i}", name=f"kt{i}") for i in range(4)]
            vaug = [pA.tile([128, HPC * (Hd + 1)], BF16, tag=f"va{i}", name=f"va{i}")
                    for i in range(NT)]
            for st in range(NT):
                ones_v = vaug[st][:, :].rearrange("p (g c) -> p g c", g=HPC)[:, :, Hd:Hd + 1]
                nc.vector.memset(ones_v, 1.0)
            yt = [pA.tile([128, S], BF16, tag=f"yt{i}", name=f"yt{i}") for i in range(2)]

            # ---- main interleaved phase ----
            with tc.tile_pool(name="scr", bufs=1, space="PSUM") as pScr, \
                 tc.tile_pool(name="ps_s", bufs=3, space="PSUM") as pPs, \
                 tc.tile_pool(name="ps_o", bufs=1, space="PSUM") as pPo, \
                 tc.tile_pool(name="qstage", bufs=3) as pQs, \
                 tc.tile_pool(name="trot", bufs=2) as pT, \
                 tc.tile_pool(name="etile", bufs=12) as pE, \
                 tc.tile_pool(name="owsb", bufs=2) as pOw, \
                 tc.tile_pool(name="gn", bufs=2) as pG, \
                 tc.tile_pool(name="outsb", bufs=2) as pOut:

                uid = [0]

                def scr(shape, dtype, nm):
                    uid[0] += 1
                    return pScr.tile(shape, dtype, tag="scr", name=f"{nm}{uid[0]}")

                # ---- feed micro-units ----
                def v_unit(st):
                    psv = scr([128, THDC], F32, f"psv{st}")
                    for i in range(DK):
                        dk = (i + st) % DK
                        nc.tensor.matmul(
                            psv,
                            lhsT=xt(dk, st // 4)[:, (st % 4) * 128:(st % 4 + 1) * 128],
                            rhs=wv_sl(dk),
                            start=(i == 0), stop=(i == DK - 1))
                    va = vaug[st]
                    vav = va[:, :].rearrange("p (g c) -> p g c", g=HPC)[:, :, 0:Hd]
                    nc.vector.tensor_copy(
                        out=vav, in_=psv[:, :].rearrange("p (h c) -> p h c", h=HPC))

                kq_state = {}

                def kq_a(which, tt_i, c):
                    psq = scr([128, 512], F32, f"psq{which}{tt_i}{c}")
                    for i in range(4):
                        dk = (i + tt_i * 2 + c) % DK
                        nc.tensor.matmul(psq, lhsT=wkq_sl(which, tt_i, dk),
                                         rhs=xt(dk, c), start=(i == 0), stop=False)
                    kq_state[(which, tt_i, c)] = psq

                def kq_b(which, tt_i, c):
                    dst = kt[tt_i] if which == 0 else qt[tt_i]
                    cs = slice(c * 512, (c + 1) * 512)
                    psq = kq_state.pop((which, tt_i, c))
                    for i in range(4, DK):
                        dk = (i + tt_i * 2 + c) % DK
                        nc.tensor.matmul(psq, lhsT=wkq_sl(which, tt_i, dk),
                                         rhs=xt(dk, c), start=False, stop=(i == DK - 1))
                    qstage = pQs.tile([128, 512], BF16, tag="qstage", name=f"qst{which}{tt_i}{c}")
                    nc.vector.tensor_copy(out=qstage, in_=psq)
                    perm = scr([128, 512], F32, f"perm{which}{tt_i}{c}")
                    nc.tensor.matmul(perm, lhsT=rotm, rhs=qstage, start=True, stop=True)
                    nc.vector.tensor_tensor(out=dst[:, cs], in0=qstage, in1=cos_sl(c), op=MULT)
                    trot = pT.tile([128, 512], BF16, tag="trot", name=f"trot{which}{tt_i}{c}")
                    nc.vector.tensor_tensor(out=trot, in0=perm, in1=sin_sl(c), op=MULT)
                    nc.vector.tensor_tensor(out=dst[:, cs], in0=dst[:, cs], in1=trot, op=ADD)

                def kq(which, tt_i, c):
                    return [lambda: kq_a(which, tt_i, c), lambda: kq_b(which, tt_i, c)]

                ost_hold = {}

                def o_half(c, st, oc):
                    stg = c * 4 + st
                    pout = scr([128, 512], F32, f"pout{stg}{oc}")
                    for ci in range(2):
                        nc.tensor.matmul(
                            pout,
                            lhsT=yt[ci][:, stg * 128:(stg + 1) * 128],
                            rhs=wo_sl(ci, oc),
                            start=(ci == 0), stop=(ci == 1))
                    if oc == 0:
                        ost_hold[stg] = pOut.tile([128, 1024], F32, tag="ost",
                                                  name=f"ost{stg}")
                    ost = ost_hold[stg]
                    nc.vector.tensor_copy(out=ost[:, oc * 512:(oc + 1) * 512], in_=pout)
                    if oc == 1:
                        del ost_hold[stg]
                        dma_sp(out=y_d[stg * 128:(stg + 1) * 128, :], in_=ost)

                def o_unit(c, st):
                    return [lambda: o_half(c, st, 0), lambda: o_half(c, st, 1)]

                # ---- attention ----
                def qk_exp(c, h, sk):
                    tt_pair = (h // 2, 2 + h // 2)
                    ro = (h % 2) * 64
                    cs = slice(c * 512, (c + 1) * 512)
                    es = []
                    for br in range(2):
                        tt_i = tt_pair[br]
                        pss = pPs.tile([128, 512], F32, tag="pss", name=f"pss{c}{h}{sk}{br}")
                        nc.tensor.matmul(pss,
                                         lhsT=kt[tt_i][ro:ro + 64, sk * 128:(sk + 1) * 128],
                                         rhs=qt[tt_i][ro:ro + 64, cs],
                                         start=True, stop=True)
                        e = pE.tile([128, 512], BF16, tag="e", name=f"e{c}{h}{sk}{br}")
                        nc.scalar.activation(out=e, in_=pss, func=AF.Exp, scale=QK_SCALE)
                        es.append(e)
                    return es

                def pv(c, h, sk, po, es):
                    # po is zeroed by a DVE memset at block start: four
                    # interleaved accumulation regions share one PSUM bank and
                    # start=True resets the whole bank, so never use it here.
                    va = vaug[sk][:, h * (Hd + 1):(h + 1) * (Hd + 1)]
                    for br in range(2):
                        for q4 in range(4):
                            nc.tensor.matmul(
                                po[br][:, q4 * 65:(q4 + 1) * 65],
                                lhsT=es[br][:, q4 * 128:(q4 + 1) * 128],
                                rhs=va,
                                start=False, stop=(sk == NT - 1),
                                skip_group_check=True)

                def gn_front(c, h, po):
                    ow1 = pOw.tile([128, HPC * (Hd + 1)], F32, tag="ow1", name=f"ow1_{c}{h}")
                    ow2 = pOw.tile([128, HPC * (Hd + 1)], F32, tag="ow2", name=f"ow2_{c}{h}")
                    nc.vector.tensor_copy(out=ow1, in_=po[0])
                    nc.vector.tensor_copy(out=ow2, in_=po[1])
                    rec = pG.tile([128, 4], F32, tag="rec", name=f"rec{c}{h}")
                    nc.vector.reciprocal(rec, _rview(ow2))
                    rho = pG.tile([128, 4], F32, tag="rho", name=f"rho{c}{h}")
                    nc.vector.tensor_tensor(out=rho, in0=_rview(ow1), in1=rec, op=MULT)
                    nc.vector.tensor_tensor(out=rho, in0=rho, in1=lam_bh[h], op=MULT)
                    dt_ = pG.tile([128, THDC], F32, tag="dt", name=f"dt{c}{h}")
                    dtv = dt_[:, :].rearrange("p (g d) -> p g d", g=4)
                    nc.vector.tensor_tensor(out=dtv, in0=_hview(ow2), in1=_b64(rho), op=MULT)
                    nc.vector.tensor_tensor(out=dtv, in0=_hview(ow1), in1=dtv, op=SUB)
                    s1 = pG.tile([128, 4], F32, tag="s1", name=f"s1{c}{h}")
                    nc.vector.reduce_sum(out=s1, in_=dtv, axis=AX)
                    nc.vector.tensor_scalar_mul(s1, s1, -1.0 / Hd)
                    nc.vector.tensor_tensor(out=dtv, in0=dtv, in1=_b64(s1), op=ADD)
                    d2 = pG.tile([128, THDC], F32, tag="d2", name=f"d2{c}{h}")
                    nc.gpsimd.tensor_tensor(out=d2, in0=dt_, in1=dt_, op=MULT)
                    s2 = pG.tile([128, 4], F32, tag="s2", name=f"s2{c}{h}")
                    nc.vector.reduce_sum(out=s2,
                                         in_=d2[:, :].rearrange("p (g d) -> p g d", g=4),
                                         axis=AX)
                    nc.vector.tensor_tensor(out=rec, in0=_rview(ow1), in1=_rview(ow1), op=MULT)
                    nc.vector.tensor_scalar_mul(rec, rec, GN_EPS)
                    nc.vector.tensor_scalar_mul(s2, s2, 1.0 / Hd)
                    nc.vector.tensor_tensor(out=s2, in0=s2, in1=rec, op=ADD)
                    return dt_, s2

                def gn_back(c, h, dt_, s2):
                    dtv = dt_[:, :].rearrange("p (g d) -> p g d", g=4)
                    nc.scalar.activation(out=s2, in_=s2, func=AF.Sqrt)
                    nc.vector.reciprocal(s2, s2)
                    nc.vector.tensor_tensor(out=dtv, in0=dtv, in1=_b64(s2), op=MULT)
                    gw = gnw_eff[:, h * Hd:(h + 1) * Hd].unsqueeze(1) \
                        .broadcast_to((128, 4, Hd))
                    gb = gnb_eff[:, h * Hd:(h + 1) * Hd].unsqueeze(1) \
                        .broadcast_to((128, 4, Hd))
                    dtb = pG.tile([128, THDC], BF16, tag="dtb", name=f"dtb{c}{h}")
                    dtbv = dtb[:, :].rearrange("p (g d) -> p g d", g=4)
                    nc.gpsimd.tensor_tensor(out=dtv, in0=dtv, in1=gw, op=MULT)
                    nc.gpsimd.tensor_tensor(out=dtbv, in0=dtv, in1=gb, op=ADD)
                    ro = (h % 2) * 64
                    pt2 = scr([64, 512], BF16, f"pt2_{c}{h}")
                    for q4 in range(4):
                        nc.tensor.transpose(pt2[:, q4 * 128:(q4 + 1) * 128],
                                            dtb[:, q4 * Hd:(q4 + 1) * Hd], ident)
                    nc.vector.tensor_copy(
                        out=yt[h // 2][ro:ro + 64, c * 512:(c + 1) * 512], in_=pt2)

                # Block schedule -----------------------------------------
                blocks = [(c, h) for c in range(NCH) for h in (0, 1)] + \
                         [(c, h) for c in range(NCH) for h in (2, 3)]
                feeds = {i: {} for i in range(16)}

                def place(bi, slot, thunks):
                    feeds[bi].setdefault(slot, []).extend(thunks)

                place(0, 1, kq(0, 0, 1))
                place(0, 2, kq(0, 2, 1))
                place(0, 3, [lambda: v_unit(4), lambda: v_unit(5), lambda: v_unit(6)])
                place(0, 4, [lambda: v_unit(7)])
                place(0, 5, kq(0, 0, 2))
                place(0, 6, kq(0, 2, 2))
                place(0, 7, [lambda: v_unit(8), lambda: v_unit(9)])
                place(0, 8, [lambda: v_unit(10), lambda: v_unit(11)])
                place(0, 9, kq(0, 0, 3))
                place(0, 10, kq(0, 2, 3))
                place(0, 11, [lambda: v_unit(12), lambda: v_unit(13)])
                place(0, 12, [lambda: v_unit(14), lambda: v_unit(15)])
                _units = {
                    1: [kq(1, 0, 1), kq(1, 2, 1), kq(0, 1, 0)],
                    2: [kq(1, 0, 2), kq(1, 2, 2), kq(0, 1, 1)],
                    3: [kq(0, 1, 2), kq(0, 1, 3), kq(0, 3, 0)],
                    4: [kq(1, 0, 3), kq(1, 2, 3), kq(0, 3, 1)],
                    5: [kq(0, 3, 2), kq(0, 3, 3)],
                    6: [kq(1, 1, 0), kq(1, 3, 0)],
                    7: [kq(1, 1, 1), kq(1, 3, 1)],
                    8: [kq(1, 1, 2), kq(1, 3, 2)],
                    9: [kq(1, 1, 3), kq(1, 3, 3)],
                    10: [o_unit(0, 0), o_unit(0, 1)],
                    11: [o_unit(0, 2), o_unit(0, 3)],
                    12: [o_unit(1, 0), o_unit(1, 1)],
                    13: [o_unit(1, 2), o_unit(1, 3)],
                    14: [o_unit(2, 0), o_unit(2, 1)],
                    15: [o_unit(2, 2), o_unit(2, 3)],
                }
                for bi, units in _units.items():
                    flat = [t for u in units for t in u]
                    # o_units (blocks >= 10) read yt written by a gn_back that
                    # lands at slot 5 of the same block -- keep them later.
                    base_slots = (1, 3, 5, 7, 9, 11) if bi < 10 else (7, 9, 11, 13)
                    nb = len(base_slots)
                    for j, t in enumerate(flat):
                        place(bi, base_slots[j % nb] + (j // nb), [t])

                # PE p-state warmup: ~10us of dummy matmuls so the clock is
                # at 2.4GHz when the first projection lands (outputs unread).
                for wi in range(48):
                    pwu = pPs.tile([128, 512], F32, tag="pss", name=f"warm{wi}")
                    nc.tensor.matmul(pwu, lhsT=dummy[:, 0:128], rhs=dummy,
                                     start=True, stop=True)
                # prologue
                for th in kq(0, 0, 0) + kq(0, 2, 0) + kq(1, 0, 0) + kq(1, 2, 0):
                    th()
                for st in range(4):
                    v_unit(st)

                from collections import deque
                LAG = 3
                stream = [(bi, c, h, sk)
                          for bi, (c, h) in enumerate(blocks) for sk in range(NT)]
                pvq = deque()
                po_cur = {}
                back_at = {}

                def pump():
                    bi, c, h, sk, po, es = pvq.popleft()
                    pv(c, h, sk, po, es)
                    if sk == NT - 1:
                        dt_, s2 = gn_front(c, h, po)
                        return (c, h, dt_, s2)
                    return None

                for idx, (bi, c, h, sk) in enumerate(stream):
                    if sk == 0:
                        par = (bi % 2) * 2
                        po_cur[bi] = [pPo.tile([128, 4 * 65], F32,
                                               tag=f"po{par + br}",
                                               name=f"po{c}{h}{br}")
                                      for br in range(2)]
                        for t in po_cur[bi]:
                            nc.vector.memset(t, 0.0)
                    es = qk_exp(c, h, sk)
                    pvq.append((bi, c, h, sk, po_cur[bi], es))
                    if len(pvq) > LAG:
                        gn = pump()
                        if gn is not None:
                            back_at[idx + 3] = gn
                    if idx in back_at:
                        gn_back(*back_at.pop(idx))
                    for f in feeds[bi].get(sk, []):
                        f()
                while pvq:
                    gn = pump()
                    if gn is not None:
                        back_at[0] = gn
                for gn in back_at.values():
                    gn_back(*gn)
                for th in o_unit(3, 0) + o_unit(3, 1) + o_unit(3, 2) + o_unit(3, 3):
                    th()

    _split_multi_waits(nc)
    return nc


_CACHE = {}


def _get_module():
    if "nc" not in _CACHE:
        _CACHE["nc"] = build_module()
        _CACHE["tables"] = _rope_tables()
    return _CACHE["nc"], _CACHE["tables"]


def _pack_dk(arr):
    """[1024, W] -> [128, 8*W] with dk blocks side by side."""
    W = arr.shape[1]
    return np.ascontiguousarray(
        arr.reshape(DK, 128, W).transpose(1, 0, 2).reshape(128, DK * W))


def kernel(x, Wq, Wk, Wv, Wo, lambda_q1, lambda_k1, lambda_q2, lambda_k2,
           lambda_init, gn_weight, gn_bias):
    import ml_dtypes
    from concourse.bass_utils import run_bass_kernel_spmd
    BF = ml_dtypes.bfloat16

    x = np.asarray(x, dtype=np.float32)
    Wq = np.asarray(Wq, dtype=np.float32).astype(BF)
    Wk = np.asarray(Wk, dtype=np.float32).astype(BF)
    Wv = np.asarray(Wv, dtype=np.float32).astype(BF)
    Wo = np.asarray(Wo, dtype=np.float32).astype(BF)
    lq1 = np.asarray(lambda_q1, dtype=np.float32)
    lk1 = np.asarray(lambda_k1, dtype=np.float32)
    lq2 = np.asarray(lambda_q2, dtype=np.float32)
    lk2 = np.asarray(lambda_k2, dtype=np.float32)
    lam_init = np.float32(np.asarray(lambda_init).reshape(()))
    gnw = np.asarray(gn_weight, dtype=np.float32)
    gnb = np.asarray(gn_bias, dtype=np.float32)

    nc, (cosT, ssinT) = _get_module()
    # 128-row, chunked rope tables (duplicated halves), bf16
    cos2h = np.concatenate([cosT, cosT], axis=0).astype(BF)   # (128, S)
    sin2h = np.concatenate([ssinT, ssinT], axis=0).astype(BF)
    rotm = _rotm().astype(BF)
    ident = np.eye(128, dtype=np.float32).astype(BF)

    in_maps = []
    for core in range(8):
        b = core // 4
        hb = (core % 4) * HPC
        c1 = slice(hb * Hd, (hb + HPC) * Hd)
        c2 = slice(H * Hd + hb * Hd, H * Hd + (hb + HPC) * Hd)
        wk_core = np.concatenate([Wk[:, c1], Wk[:, c2]], axis=1)  # (1024, 512)
        wq_core = np.concatenate([Wq[:, c1], Wq[:, c2]], axis=1)
        xT = np.ascontiguousarray(x[b].astype(BF).T)              # (1024, 2048)

        def tt_pack(w, tts):
            # (1024, 512) -> [128, 8*(128*len(tts))] with dk blocks of [tt...]
            cols = np.concatenate([w[:, t * 128:(t + 1) * 128] for t in tts], axis=1)
            return _pack_dk(cols)

        b1 = np.concatenate([
            _pack_dk(xT[:, 0:512]),
            tt_pack(wk_core, (0, 2)),
            tt_pack(wq_core, (0, 2)),
            cos2h[:, 0:512], sin2h[:, 0:512],
            rotm, ident,
        ], axis=1)
        bcs = []
        for c in (1, 2, 3):
            bcs.append(np.concatenate([
                _pack_dk(xT[:, c * 512:(c + 1) * 512]),
                cos2h[:, c * 512:(c + 1) * 512],
                sin2h[:, c * 512:(c + 1) * 512],
            ], axis=1))
        lam1 = np.exp(lq1[hb:hb + HPC] * lk1[hb:hb + HPC])
        lam2 = np.exp(lq2[hb:hb + HPC] * lk2[hb:hb + HPC])
        lam_full = (lam1 - lam2 + lam_init).astype(np.float32)      # (4,)
        slam = np.float32(1.0) - lam_init
        gnw_row = (gnw[c1] * slam).astype(BF)
        gnb_row = (gnb[c1] * slam).astype(BF)
        lam_row = np.repeat(lam_full, 4).astype(BF)                 # (16,)
        b5 = np.concatenate([
            tt_pack(wk_core, (1, 3)),
            tt_pack(wq_core, (1, 3)),
            np.ascontiguousarray(
                Wo[c1, :].reshape(2, 128, D).transpose(1, 0, 2).reshape(128, 2 * D)),
            np.broadcast_to(gnw_row, (128, THDC)),
            np.broadcast_to(gnb_row, (128, THDC)),
            np.broadcast_to(lam_row, (128, 16)),
        ], axis=1)
        in_maps.append({
            "b1": np.ascontiguousarray(b1),
            "wvb": _pack_dk(Wv[:, c1]),
            "bc1": np.ascontiguousarray(bcs[0]),
            "bc2": np.ascontiguousarray(bcs[1]),
            "bc3": np.ascontiguousarray(bcs[2]),
            "b5": np.ascontiguousarray(b5),
        })

    last_err = None
    for attempt in range(3):
        try:
            res = run_bass_kernel_spmd(nc, in_maps, core_ids=list(range(8)))
            break
        except Exception as e:  # transient axon/device hiccups
            last_err = e
            time.sleep(10 * (attempt + 1))
    else:
        raise last_err

    out = np.zeros((B, S, D), dtype=np.float32)
    for core in range(8):
        out[core // 4] += res.results[core]["y"]
    return out


# revision 30
# speedup vs baseline: 1.0406x; 1.0406x over previous
"""Differential attention (nn_DifferentialAttention_84679575208071) on 8 TRN2
NeuronCores via Bass/Tile.

Sharding: hybrid data-parallel x tensor-parallel. Core c handles batch c//4 and
heads 4*(c%4) .. 4*(c%4)+4 (B=2, H=16 -> 2 batch groups x 4-way head split).
Each core computes its 4 heads' attention + group-norm + partial o_proj; the
host sums the 4 partial (S, D) outputs per batch and stacks the 2 batches.

v5 design (cost-model driven):
  - The host pre-transposes x and packs ALL inputs (x^T, weights, rope tables,
    identity/rotation matrices) into 8 need-ordered bf16 column bundles, one
    DMA each.  Every DMA costs ~3us of queue-head ring-semaphore blocking, so
    transfer count -- not bytes -- dominates the prologue; with 8 bundles the
    first exp fires at ~11us instead of ~59us.
  - Attention PV matmuls reoriented: E (bf16, SBUF) is the stationary operand,
    aug-V [64 cols + ones] the moving one -> 65-column matmuls, and the output
    lands token-major [q, ch], which is what group-norm wants.
  - Per-(sk, branch) pipeline: QK [128,512] -> exp -> 4 PV through a PSUM
    ring; the next block's first QK/exp pair is emitted ahead of the current
    block's last PV so the ACT engine (the ~220us exp floor) never waits
    across block boundaries.
  - PV accumulators double-buffered by block parity so a new block's PV
    matmuls never wait on the previous block's group-norm drain.
  - Projections and o_proj split into half-units and spread over 14 per-block
    feed slots so PE bursts never outrun the exp buffer.
"""
import json
import time

import numpy as np

import bass_rust
import concourse.bass as bass
import concourse.mybir as mybir
import concourse.tile as tile

F32 = mybir.dt.float32
BF16 = mybir.dt.bfloat16
MULT = mybir.AluOpType.mult
ADD = mybir.AluOpType.add
SUB = mybir.AluOpType.subtract
AX = mybir.AxisListType.X
AF = mybir.ActivationFunctionType

B, S, D = 2, 2048, 1024
H, Hd = 16, 64
HPC = 4               # heads per core
THDC = HPC * Hd       # 256 channels per core
NT = S // 128         # 16 s-tiles
NCH = S // 512        # 4 chunks
DK = D // 128         # 8 contraction tiles
LAMBDA_INIT = 0.8
GN_EPS = 1e-5
ROPE_BASE = 10000.0
QK_SCALE = float(Hd) ** -0.5  # 0.125

# bund1 column offsets (bf16): xT-c0 | wk02 | wq02 | cos0 | sin0 | rotm | ident
B1_XT = 0             # 8 dk x 512
B1_WK = 4096          # 8 dk x (tt0 128 | tt2 128)
B1_WQ = 6144
B1_COS = 8192
B1_SIN = 8704
B1_ROTM = 9216
B1_ID = 9344
B1_W = 9472
# bundc (c=1,2,3): xT-c | cos-c | sin-c
BC_COS = 4096
BC_SIN = 4608
BC_W = 5120
# bund5: wk13 | wq13 | wo | gnw_eff | gnb_eff | lam_bh
B5_WK = 0
B5_WQ = 2048
B5_WO = 4096
B5_GNW = 6144
B5_GNB = 6400
B5_LAM = 6656
B5_W = 6672


def _split_multi_waits(nc):
    """This container's walrus rejects >1 sync wait per instruction. Hoist
    extra waits onto same-engine NoOps inserted right before the instruction
    (engine queues are in-order, so semantics are unchanged)."""
    d = json.loads(bass_rust.module_to_json_string(nc.m))
    ctr = 0
    for f in d["functions"]:
        for bb in f["blocks"]:
            out = []
            for inst in bb.get("instructions", []):
                si = inst.get("sync_info")
                waits = si.get("on_wait", []) if si else []
                if len(waits) > 1:
                    for w in waits[:-1]:
                        ctr += 1
                        out.append({
                            "debug": inst.get("debug", 0),
                            "engine": inst["engine"],
                            "ins": [],
                            "name": f"WSPLIT-{ctr}",
                            "opcode": "NoOp",
                            "outs": [],
                            "sync_info": {"on_update": [], "on_wait": [w]},
                        })
                    si["on_wait"] = [waits[-1]]
                out.append(inst)
            bb["instructions"] = out
    nc.m = bass_rust.module_from_json_string(json.dumps(d))


def _rope_tables():
    inv_freq = 1.0 / (ROPE_BASE ** (np.arange(0, Hd, 2, dtype=np.float32) / Hd))
    t = np.arange(S, dtype=np.float32)
    freqs = np.outer(t, inv_freq).astype(np.float32)
    emb = np.concatenate([freqs, freqs], axis=-1)       # (S, Hd)
    cos = np.cos(emb).astype(np.float32)
    sin = np.sin(emb).astype(np.float32)
    return cos.T.copy(), sin.T.copy()                   # (Hd, S) each


def _rotm():
    m64 = np.zeros((64, 64), dtype=np.float32)
    for d_ in range(32):
        m64[d_, d_ + 32] = -1.0
        m64[d_ + 32, d_] = 1.0
    m128 = np.zeros((128, 128), dtype=np.float32)
    m128[:64, :64] = m64
    m128[64:, 64:] = m64
    return np.ascontiguousarray(m128.T)


def _hview(t):
    return t[:, :].rearrange("p (g c) -> p g c", g=HPC)[:, :, 0:Hd]


def _rview(t):
    v = t[:, :].rearrange("p (g c) -> p g c", g=HPC)[:, :, Hd:Hd + 1]
    return v.rearrange("p g c -> p (g c)")


def _b64(ap):
    return ap.unsqueeze(2).broadcast_to((128, HPC, Hd))


def build_module():
    nc = bass.Bass(trn_type="TRN2")

    b1_d = nc.dram_tensor("b1", [128, B1_W], BF16, kind="ExternalInput")
    wv_d = nc.dram_tensor("wvb", [128, 2048], BF16, kind="ExternalInput")
    bc_d = [nc.dram_tensor(f"bc{c}", [128, BC_W], BF16, kind="ExternalInput")
            for c in (1, 2, 3)]
    b5_d = nc.dram_tensor("b5", [128, B5_W], BF16, kind="ExternalInput")
    y_d = nc.dram_tensor("y", [S, D], F32, kind="ExternalOutput")

    dma_sp = nc.sync.dma_start

    with tile.TileContext(nc) as tc:
        with tc.tile_pool(name="persist", bufs=1) as pA:
            # ---- 8 input DMAs on the SP queue, in need order ----
            b1 = pA.tile([128, B1_W], BF16, tag="b1", name="b1")
            dma_sp(out=b1, in_=b1_d.ap())
            wvb = pA.tile([128, 2048], BF16, tag="wvb", name="wvb")
            dma_sp(out=wvb, in_=wv_d.ap())
            bc = [None]
            for i in range(3):
                t = pA.tile([128, BC_W], BF16, tag=f"bc{i}", name=f"bc{i}")
                dma_sp(out=t, in_=bc_d[i].ap())
                bc.append(t)
            b5 = pA.tile([128, B5_W], BF16, tag="b5", name="b5")
            dma_sp(out=b5, in_=b5_d.ap())

            rotm = b1[:, B1_ROTM:B1_ROTM + 128]
            ident = b1[:, B1_ID:B1_ID + 128]

            def xt(dk, c):
                t = b1 if c == 0 else bc[c]
                return t[:, dk * 512:(dk + 1) * 512]

            def wkq_sl(which, tt, dk):
                # which 0=K 1=Q
                if tt in (0, 2):
                    base = (B1_WK if which == 0 else B1_WQ)
                    return b1[:, base + dk * 256 + (tt // 2) * 128:
                              base + dk * 256 + (tt // 2) * 128 + 128]
                base = (B5_WK if which == 0 else B5_WQ)
                return b5[:, base + dk * 256 + (tt // 2) * 128:
                          base + dk * 256 + (tt // 2) * 128 + 128]

            def wv_sl(dk):
                return wvb[:, dk * THDC:(dk + 1) * THDC]

            def wo_sl(ci, oc):
                return b5[:, B5_WO + ci * 1024 + oc * 512:B5_WO + ci * 1024 + (oc + 1) * 512]

            def cos_sl(c):
                t = b1 if c == 0 else bc[c]
                off = B1_COS if c == 0 else BC_COS
                return t[:, off:off + 512]

            def sin_sl(c):
                t = b1 if c == 0 else bc[c]
                off = B1_SIN if c == 0 else BC_SIN
                return t[:, off:off + 512]

            # lambda / group-norm affine constants are host-precomputed
            # bf16 broadcast rows packed into b5.
            gnw_eff = b5[:, B5_GNW:B5_GNW + THDC]
            gnb_eff = b5[:, B5_GNB:B5_GNB + THDC]
            lam_bh = [b5[:, B5_LAM + 4 * h:B5_LAM + 4 * (h + 1)] for h in range(HPC)]

            # persistent compute tensors
            dummy = pA.tile([128, 512], BF16, tag="dummy", name="dummy")
            nc.vector.memset(dummy, 0.5)
            qt = [pA.tile([128, S], BF16, tag=f"qt{i}", name=f"qt{i}") for i in range(4)]
            kt = [pA.tile([128, S], BF16, tag=f"kt{i}", name=f"kt{i}") for i in range(4)]
            vaug = [pA.tile([128, HPC * (Hd + 1)], BF16, tag=f"va{i}", name=f"va{i}")
                    for i in range(NT)]
            for st in range(NT):
                ones_v = vaug[st][:, :].rearrange("p (g c) -> p g c", g=HPC)[:, :, Hd:Hd + 1]
                nc.vector.memset(ones_v, 1.0)
            yt = [pA.tile([128, S], BF16, tag=f"yt{i}", name=f"yt{i}") for i in range(2)]

            # ---- main interleaved phase ----
            with tc.tile_pool(name="scr", bufs=2, space="PSUM") as pScr, \
                 tc.tile_pool(name="ps_s", bufs=4, space="PSUM") as pPs, \
                 tc.tile_pool(name="ps_o", bufs=1, space="PSUM") as pPo, \
                 tc.tile_pool(name="qstage", bufs=3) as pQs, \
                 tc.tile_pool(name="trot", bufs=2) as pT, \
                 tc.tile_pool(name="etile", bufs=12) as pE, \
                 tc.tile_pool(name="owsb", bufs=2) as pOw, \
                 tc.tile_pool(name="gn", bufs=2) as pG, \
                 tc.tile_pool(name="outsb", bufs=2) as pOut:

                uid = [0]

                def scr(shape, dtype, nm):
                    uid[0] += 1
                    return pScr.tile(shape, dtype, tag="scr", name=f"{nm}{uid[0]}")

                # ---- feed micro-units ----
                def v_unit(st):
                    psv = scr([128, THDC], F32, f"psv{st}")
                    for i in range(DK):
                        dk = (i + st) % DK
                        nc.tensor.matmul(
                            psv,
                            lhsT=xt(dk, st // 4)[:, (st % 4) * 128:(st % 4 + 1) * 128],
                            rhs=wv_sl(dk),
                            start=(i == 0), stop=(i == DK - 1))
                    va = vaug[st]
                    vav = va[:, :].rearrange("p (g c) -> p g c", g=HPC)[:, :, 0:Hd]
                    nc.vector.tensor_copy(
                        out=vav, in_=psv[:, :].rearrange("p (h c) -> p h c", h=HPC))

                kq_state = {}

                def kq_a(which, tt_i, c):
                    psq = scr([128, 512], F32, f"psq{which}{tt_i}{c}")
                    for i in range(4):
                        dk = (i + tt_i * 2 + c) % DK
                        nc.tensor.matmul(psq, lhsT=wkq_sl(which, tt_i, dk),
                                         rhs=xt(dk, c), start=(i == 0), stop=False)
                    kq_state[(which, tt_i, c)] = psq

                def kq_b(which, tt_i, c):
                    dst = kt[tt_i] if which == 0 else qt[tt_i]
                    cs = slice(c * 512, (c + 1) * 512)
                    psq = kq_state.pop((which, tt_i, c))
                    for i in range(4, DK):
                        dk = (i + tt_i * 2 + c) % DK
                        nc.tensor.matmul(psq, lhsT=wkq_sl(which, tt_i, dk),
                                         rhs=xt(dk, c), start=False, stop=(i == DK - 1))
                    qstage = pQs.tile([128, 512], BF16, tag="qstage", name=f"qst{which}{tt_i}{c}")
                    nc.vector.tensor_copy(out=qstage, in_=psq)
                    perm = scr([128, 512], F32, f"perm{which}{tt_i}{c}")
                    nc.tensor.matmul(perm, lhsT=rotm, rhs=qstage, start=True, stop=True)
                    nc.vector.tensor_tensor(out=dst[:, cs], in0=qstage, in1=cos_sl(c), op=MULT)
                    trot = pT.tile([128, 512], BF16, tag="trot", name=f"trot{which}{tt_i}{c}")
                    nc.vector.tensor_tensor(out=trot, in0=perm, in1=sin_sl(c), op=MULT)
                    nc.vector.tensor_tensor(out=dst[:, cs], in0=dst[:, cs], in1=trot, op=ADD)

                def kq(which, tt_i, c):
                    return [lambda: kq_a(which, tt_i, c), lambda: kq_b(which, tt_i, c)]

                ost_hold = {}

                def o_half(c, st, oc):
                    stg = c * 4 + st
                    pout = scr([128, 512], F32, f"pout{stg}{oc}")
                    for ci in range(2):
                        nc.tensor.matmul(
                            pout,
                            lhsT=yt[ci][:, stg * 128:(stg + 1) * 128],
                            rhs=wo_sl(ci, oc),
                            start=(ci == 0), stop=(ci == 1))
                    if oc == 0:
                        ost_hold[stg] = pOut.tile([128, 1024], F32, tag="ost",
                                                  name=f"ost{stg}")
                    ost = ost_hold[stg]
                    nc.vector.tensor_copy(out=ost[:, oc * 512:(oc + 1) * 512], in_=pout)
                    if oc == 1:
                        del ost_hold[stg]
                        dma_sp(out=y_d[stg * 128:(stg + 1) * 128, :], in_=ost)

                def o_unit(c, st):
                    return [lambda: o_half(c, st, 0), lambda: o_half(c, st, 1)]

                # ---- attention ----
                def qk_exp(c, h, sk):
                    tt_pair = (h // 2, 2 + h // 2)
                    ro = (h % 2) * 64
                    cs = slice(c * 512, (c + 1) * 512)
                    es = []
                    for br in range(2):
                        tt_i = tt_pair[br]
                        pss = pPs.tile([128, 512], F32, tag="pss", name=f"pss{c}{h}{sk}{br}")
                        nc.tensor.matmul(pss,
                                         lhsT=kt[tt_i][ro:ro + 64, sk * 128:(sk + 1) * 128],
                                         rhs=qt[tt_i][ro:ro + 64, cs],
                                         start=True, stop=True)
                        e = pE.tile([128, 512], BF16, tag="e", name=f"e{c}{h}{sk}{br}")
                        nc.scalar.activation(out=e, in_=pss, func=AF.Exp, scale=QK_SCALE)
                        es.append(e)
                    return es

                def pv(c, h, sk, po, es):
                    # po is zeroed by a DVE memset at block start: four
                    # interleaved accumulation regions share one PSUM bank and
                    # start=True resets the whole bank, so never use it here.
                    va = vaug[sk][:, h * (Hd + 1):(h + 1) * (Hd + 1)]
                    for br in range(2):
                        for q4 in range(4):
                            nc.tensor.matmul(
                                po[br][:, q4 * 65:(q4 + 1) * 65],
                                lhsT=es[br][:, q4 * 128:(q4 + 1) * 128],
                                rhs=va,
                                start=False, stop=(sk == NT - 1),
                                skip_group_check=True)

                def gn_front(c, h, po):
                    ow1 = pOw.tile([128, HPC * (Hd + 1)], F32, tag="ow1", name=f"ow1_{c}{h}")
                    ow2 = pOw.tile([128, HPC * (Hd + 1)], F32, tag="ow2", name=f"ow2_{c}{h}")
                    nc.vector.tensor_copy(out=ow1, in_=po[0])
                    nc.vector.tensor_copy(out=ow2, in_=po[1])
                    rec = pG.tile([128, 4], F32, tag="rec", name=f"rec{c}{h}")
                    nc.vector.reciprocal(rec, _rview(ow2))
                    rho = pG.tile([128, 4], F32, tag="rho", name=f"rho{c}{h}")
                    nc.vector.tensor_tensor(out=rho, in0=_rview(ow1), in1=rec, op=MULT)
                    nc.vector.tensor_tensor(out=rho, in0=rho, in1=lam_bh[h], op=MULT)
                    dt_ = pG.tile([128, THDC], F32, tag="dt", name=f"dt{c}{h}")
                    dtv = dt_[:, :].rearrange("p (g d) -> p g d", g=4)
                    nc.vector.tensor_tensor(out=dtv, in0=_hview(ow2), in1=_b64(rho), op=MULT)
                    nc.vector.tensor_tensor(out=dtv, in0=_hview(ow1), in1=dtv, op=SUB)
                    s1 = pG.tile([128, 4], F32, tag="s1", name=f"s1{c}{h}")
                    nc.vector.reduce_sum(out=s1, in_=dtv, axis=AX)
                    nc.vector.tensor_scalar_mul(s1, s1, -1.0 / Hd)
                    nc.vector.tensor_tensor(out=dtv, in0=dtv, in1=_b64(s1), op=ADD)
                    d2 = pG.tile([128, THDC], F32, tag="d2", name=f"d2{c}{h}")
                    nc.gpsimd.tensor_tensor(out=d2, in0=dt_, in1=dt_, op=MULT)
                    s2 = pG.tile([128, 4], F32, tag="s2", name=f"s2{c}{h}")
                    nc.vector.reduce_sum(out=s2,
                                         in_=d2[:, :].rearrange("p (g d) -> p g d", g=4),
                                         axis=AX)
                    nc.vector.tensor_tensor(out=rec, in0=_rview(ow1), in1=_rview(ow1), op=MULT)
                    nc.vector.tensor_scalar_mul(rec, rec, GN_EPS)
                    nc.vector.tensor_scalar_mul(s2, s2, 1.0 / Hd)
                    nc.vector.tensor_tensor(out=s2, in0=s2, in1=rec, op=ADD)
                    return dt_, s2

                def gn_back(c, h, dt_, s2):
                    dtv = dt_[:, :].rearrange("p (g d) -> p g d", g=4)
                    nc.scalar.activation(out=s2, in_=s2, func=AF.Sqrt)
                    nc.vector.reciprocal(s2, s2)
                    nc.vector.tensor_tensor(out=dtv, in0=dtv, in1=_b64(s2), op=MULT)
                    gw = gnw_eff[:, h * Hd:(h + 1) * Hd].unsqueeze(1) \
                        .broadcast_to((128, 4, Hd))
                    gb = gnb_eff[:, h * Hd:(h + 1) * Hd].unsqueeze(1) \
                        .broadcast_to((128, 4, Hd))
                    dtb = pG.tile([128, THDC], BF16, tag="dtb", name=f"dtb{c}{h}")
                    dtbv = dtb[:, :].rearrange("p (g d) -> p g d", g=4)
                    nc.gpsimd.tensor_tensor(out=dtv, in0=dtv, in1=gw, op=MULT)
                    nc.gpsimd.tensor_tensor(out=dtbv, in0=dtv, in1=gb, op=ADD)
                    ro = (h % 2) * 64
                    pt2 = scr([64, 512], BF16, f"pt2_{c}{h}")
                    for q4 in range(4):
                        nc.tensor.transpose(pt2[:, q4 * 128:(q4 + 1) * 128],
                                            dtb[:, q4 * Hd:(q4 + 1) * Hd], ident)
                    nc.vector.tensor_copy(
                        out=yt[h // 2][ro:ro + 64, c * 512:(c + 1) * 512], in_=pt2)

                # Block schedule -----------------------------------------
                blocks = [(c, h) for c in range(NCH) for h in (0, 1)] + \
                         [(c, h) for c in range(NCH) for h in (2, 3)]
                feeds = {i: {} for i in range(16)}

                def place(bi, slot, thunks):
                    feeds[bi].setdefault(slot, []).extend(thunks)

                place(0, 1, kq(0, 0, 1))
                place(0, 2, kq(0, 2, 1))
                place(0, 3, [lambda: v_unit(4), lambda: v_unit(5), lambda: v_unit(6)])
                place(0, 4, [lambda: v_unit(7)])
                place(0, 5, kq(0, 0, 2))
                place(0, 6, kq(0, 2, 2))
                place(0, 7, [lambda: v_unit(8), lambda: v_unit(9)])
                place(0, 8, [lambda: v_unit(10), lambda: v_unit(11)])
                place(0, 9, kq(0, 0, 3))
                place(0, 10, kq(0, 2, 3))
                place(0, 11, [lambda: v_unit(12), lambda: v_unit(13)])
                place(0, 12, [lambda: v_unit(14), lambda: v_unit(15)])
                _units = {
                    1: [kq(1, 0, 1), kq(1, 2, 1), kq(0, 1, 0)],
                    2: [kq(1, 0, 2), kq(1, 2, 2), kq(0, 1, 1)],
                    3: [kq(0, 1, 2), kq(0, 1, 3), kq(0, 3, 0)],
                    4: [kq(1, 0, 3), kq(1, 2, 3), kq(0, 3, 1)],
                    5: [kq(0, 3, 2), kq(0, 3, 3)],
                    6: [kq(1, 1, 0), kq(1, 3, 0)],
                    7: [kq(1, 1, 1), kq(1, 3, 1)],
                    8: [kq(1, 1, 2), kq(1, 3, 2)],
                    9: [kq(1, 1, 3), kq(1, 3, 3)],
                    10: [o_unit(0, 0), o_unit(0, 1)],
                    11: [o_unit(0, 2), o_unit(0, 3)],
                    12: [o_unit(1, 0), o_unit(1, 1)],
                    13: [o_unit(1, 2), o_unit(1, 3)],
                    14: [o_unit(2, 0), o_unit(2, 1)],
                    15: [o_unit(2, 2), o_unit(2, 3)],
                }
                for bi, units in _units.items():
                    flat = [t for u in units for t in u]
                    # o_units (blocks >= 10) read yt written by a gn_back that
                    # lands at slot 5 of the same block -- keep them later.
                    base_slots = (1, 3, 5, 7, 9, 11) if bi < 10 else (7, 9, 11, 13)
                    nb = len(base_slots)
                    for j, t in enumerate(flat):
                        place(bi, base_slots[j % nb] + (j // nb), [t])

                # PE p-state warmup: ~10us of dummy matmuls so the clock is
                # at 2.4GHz when the first projection lands (outputs unread).
                for wi in range(48):
                    pwu = pPs.tile([128, 512], F32, tag="pss", name=f"warm{wi}")
                    nc.tensor.matmul(pwu, lhsT=dummy[:, 0:128], rhs=dummy,
                                     start=True, stop=True)
                # prologue
                for th in kq(0, 0, 0) + kq(0, 2, 0) + kq(1, 0, 0) + kq(1, 2, 0):
                    th()
                for st in range(4):
                    v_unit(st)

                from collections import deque
                LAG = 3
                stream = [(bi, c, h, sk)
                          for bi, (c, h) in enumerate(blocks) for sk in range(NT)]
                pvq = deque()
                po_cur = {}
                back_at = {}

                def pump():
                    bi, c, h, sk, po, es = pvq.popleft()
                    if sk == 0:
                        for t in po:
                            nc.vector.memset(t, 0.0)
                    pv(c, h, sk, po, es)
                    if sk == NT - 1:
                        dt_, s2 = gn_front(c, h, po)
                        return (c, h, dt_, s2)
                    return None

                for idx, (bi, c, h, sk) in enumerate(stream):
                    if sk == 0:
                        po_cur[bi] = [pPo.tile([128, 4 * 65], F32, tag=f"po{br}",
                                               name=f"po{c}{h}{br}")
                                      for br in range(2)]
                    es = qk_exp(c, h, sk)
                    pvq.append((bi, c, h, sk, po_cur[bi], es))
                    if len(pvq) > LAG:
                        gn = pump()
                        if gn is not None:
                            back_at[idx + 3] = gn
                    if idx in back_at:
                        gn_back(*back_at.pop(idx))
                    for f in feeds[bi].get(sk, []):
                        f()
                while pvq:
                    gn = pump()
                    if gn is not None:
                        back_at[0] = gn
                for gn in back_at.values():
                    gn_back(*gn)
                for th in o_unit(3, 0) + o_unit(3, 1) + o_unit(3, 2) + o_unit(3, 3):
                    th()

    _split_multi_waits(nc)
    return nc


_CACHE = {}


def _get_module():
    if "nc" not in _CACHE:
        _CACHE["nc"] = build_module()
        _CACHE["tables"] = _rope_tables()
    return _CACHE["nc"], _CACHE["tables"]


def _pack_dk(arr):
    """[1024, W] -> [128, 8*W] with dk blocks side by side."""
    W = arr.shape[1]
    return np.ascontiguousarray(
        arr.reshape(DK, 128, W).transpose(1, 0, 2).reshape(128, DK * W))


def kernel(x, Wq, Wk, Wv, Wo, lambda_q1, lambda_k1, lambda_q2, lambda_k2,
           lambda_init, gn_weight, gn_bias):
    import ml_dtypes
    from concourse.bass_utils import run_bass_kernel_spmd
    BF = ml_dtypes.bfloat16

    x = np.asarray(x, dtype=np.float32)
    Wq = np.asarray(Wq, dtype=np.float32).astype(BF)
    Wk = np.asarray(Wk, dtype=np.float32).astype(BF)
    Wv = np.asarray(Wv, dtype=np.float32).astype(BF)
    Wo = np.asarray(Wo, dtype=np.float32).astype(BF)
    lq1 = np.asarray(lambda_q1, dtype=np.float32)
    lk1 = np.asarray(lambda_k1, dtype=np.float32)
    lq2 = np.asarray(lambda_q2, dtype=np.float32)
    lk2 = np.asarray(lambda_k2, dtype=np.float32)
    lam_init = np.float32(np.asarray(lambda_init).reshape(()))
    gnw = np.asarray(gn_weight, dtype=np.float32)
    gnb = np.asarray(gn_bias, dtype=np.float32)

    nc, (cosT, ssinT) = _get_module()
    # 128-row, chunked rope tables (duplicated halves), bf16
    cos2h = np.concatenate([cosT, cosT], axis=0).astype(BF)   # (128, S)
    sin2h = np.concatenate([ssinT, ssinT], axis=0).astype(BF)
    rotm = _rotm().astype(BF)
    ident = np.eye(128, dtype=np.float32).astype(BF)

    in_maps = []
    for core in range(8):
        b = core // 4
        hb = (core % 4) * HPC
        c1 = slice(hb * Hd, (hb + HPC) * Hd)
        c2 = slice(H * Hd + hb * Hd, H * Hd + (hb + HPC) * Hd)
        wk_core = np.concatenate([Wk[:, c1], Wk[:, c2]], axis=1)  # (1024, 512)
        wq_core = np.concatenate([Wq[:, c1], Wq[:, c2]], axis=1)
        xT = np.ascontiguousarray(x[b].astype(BF).T)              # (1024, 2048)

        def tt_pack(w, tts):
            # (1024, 512) -> [128, 8*(128*len(tts))] with dk blocks of [tt...]
            cols = np.concatenate([w[:, t * 128:(t + 1) * 128] for t in tts], axis=1)
            return _pack_dk(cols)

        b1 = np.concatenate([
            _pack_dk(xT[:, 0:512]),
            tt_pack(wk_core, (0, 2)),
            tt_pack(wq_core, (0, 2)),
            cos2h[:, 0:512], sin2h[:, 0:512],
            rotm, ident,
        ], axis=1)
        bcs = []
        for c in (1, 2, 3):
            bcs.append(np.concatenate([
                _pack_dk(xT[:, c * 512:(c + 1) * 512]),
                cos2h[:, c * 512:(c + 1) * 512],
                sin2h[:, c * 512:(c + 1) * 512],
            ], axis=1))
        lam1 = np.exp(lq1[hb:hb + HPC] * lk1[hb:hb + HPC])
        lam2 = np.exp(lq2[hb:hb + HPC] * lk2[hb:hb + HPC])
        lam_full = (lam1 - lam2 + lam_init).astype(np.float32)      # (4,)
        slam = np.float32(1.0) - lam_init
        gnw_row = (gnw[c1] * slam).astype(BF)
        gnb_row = (gnb[c1] * slam).astype(BF)
        lam_row = np.repeat(lam_full, 4).astype(BF)                 # (16,)
        b5 = np.concatenate([
            tt_pack(wk_core, (1, 3)),
            tt_pack(wq_core, (1, 3)),
            np.ascontiguousarray(
                Wo[c1, :].reshape(2, 128, D).transpose(1, 0, 2).reshape(128, 2 * D)),
            np.broadcast_to(gnw_row, (128, THDC)),
            np.broadcast_to(gnb_row, (128, THDC)),
            np.broadcast_to(lam_row, (128, 16)),
        ], axis=1)
        in_maps.append({
            "b1": np.ascontiguousarray(b1),
            "wvb": _pack_dk(Wv[:, c1]),
            "bc1": np.ascontiguousarray(bcs[0]),
            "bc2": np.ascontiguousarray(bcs[1]),
            "bc3": np.ascontiguousarray(bcs[2]),
            "b5": np.ascontiguousarray(b5),
        })

    last_err = None
    for attempt in range(3):
        try:
            res = run_bass_kernel_spmd(nc, in_maps, core_ids=list(range(8)))
            break
        except Exception as e:  # transient axon/device hiccups
            last_err = e
            time.sleep(10 * (attempt + 1))
    else:
        raise last_err

    out = np.zeros((B, S, D), dtype=np.float32)
    for core in range(8):
        out[core // 4] += res.results[core]["y"]
    return out
